# revision 14
# baseline (speedup 1.0000x reference)
"""Multi-head dot-product attention (B=2, S=2048, F=1024, H=16, DH=64, O=1024)
as a Bass/Tile kernel on 8 Trainium2 NeuronCores.

Sharding: data-parallel over B (2) x tensor-parallel over H (4 groups of 4
heads) = 8 cores. Each core computes q/k/v projections for its 4 heads,
softmax attention, and a partial output projection; the host sums the 4
partial outputs per batch element and adds the bias.

Device layouts (per core):
  xqT, xkvT  [F, S]  fp16   host-pre-transposed activations
  wq, wk, wv [F, 4*DH] fp16 weight shards (wq pre-scaled by 1/sqrt(DH))
  wo         [4*DH, O] fp16
  out        [S, O]  fp16   partial output (host accumulates in fp32)

Attention works in transposed-score space: sT[k, q] = KT_slice.T @ QT (two
heads packed into PE row-groups 0-63 / 64-127), one exp on ACT covers both
heads, then y'T = V'.T @ PT where V' carries a ones column so row 64 of y'T
accumulates the softmax denominator (scores are O(1), so max-subtraction is
unnecessary).

Pipeline (v2): the exp stream on ACT (135us total) and the matmul stream on
PE (~150us warm) are the two poles; the schedule keeps both dense:
  - chunk 0's x DMAs are split in quarters so real projection matmuls start
    ~1us into the kernel and HAM never re-throttles;
  - during the projection phase the first 3 blocks' scores+exp prefill
    (48/128 units) on the otherwise-idle ACT engine (pt tiles rotate 3-deep
    so 3 blocks can be in flight);
  - in the attention phase each (block, kt) unit runs 2 AV matmuls, pops one
    deferred normalization/out-projection item, and emits scores+exp for
    (block+3, kt) into the pt slot its own AV just freed;
  - psY PSUM tiles rotate through 3 banks and are staged out to SBUF right
    at block end, so the next block's AV matmuls never wait on PSUM;
  - the output rides fp16 DMA (host accumulates partial sums in fp32).
"""

import numpy as np

import concourse.bass as bass
import concourse.mybir as mybir
import concourse.tile as tile
from concourse import bacc
from concourse.bass_utils import run_bass_kernel_spmd

F32 = mybir.dt.float32
F16 = mybir.dt.float16
AF = mybir.ActivationFunctionType

B, S, F, H, DH, O = 2, 2048, 1024, 16, 64, 1024
NCORES = 8
HPC = 4  # heads per core
CH = 512  # q-chunk width
P = 128


def build_program(s=S, f=F, o=O, hpc=HPC):
    npair = hpc // 2
    nch = s // CH  # q chunks
    nkt = s // P  # k tiles
    nf = f // P  # contraction tiles for projections
    hd = hpc * DH  # stacked head dims per core (256)
    nblk = nch * npair

    nc = bacc.Bacc("TRN2", target_bir_lowering=False, debug=False, num_devices=NCORES)

    xqT = nc.dram_tensor("xqT", [f, s], F16, kind="ExternalInput")
    xkvT = nc.dram_tensor("xkvT", [f, s], F16, kind="ExternalInput")
    wq = nc.dram_tensor("wq", [f, hd], F16, kind="ExternalInput")
    wk = nc.dram_tensor("wk", [f, hd], F16, kind="ExternalInput")
    wv = nc.dram_tensor("wv", [f, hd], F16, kind="ExternalInput")
    wo = nc.dram_tensor("wo", [hd, o], F16, kind="ExternalInput")
    out = nc.dram_tensor("out", [s, o], F16, kind="ExternalOutput")

    xqT_t = xqT.ap().rearrange("(t p) n -> p t n", p=P)  # [128, nf, s]
    xkvT_t = xkvT.ap().rearrange("(t p) n -> p t n", p=P)
    wq_t = wq.ap().rearrange("(t p) n -> p t n", p=P)  # [128, nf, hd]
    wk_t = wk.ap().rearrange("(t p) n -> p t n", p=P)
    wv_t = wv.ap().rearrange("(t p) n -> p t n", p=P)
    wo_t = wo.ap().rearrange("(t p) n -> p t n", p=P)  # [128, hd//128, o]

    # per-chunk prefill schedule: after chunk c's Q/K/V projections, these
    # (block, kt) units' scores+exp are emitted (feasible: block's q-chunk
    # and kt//4 are both <= c; capacity: 3 pt parities)
    PREFILL = {
        0: [(0, k) for k in range(4)] + [(1, k) for k in range(4)],
        1: [(0, k) for k in range(4, 8)]
        + [(1, k) for k in range(4, 8)]
        + [(2, k) for k in range(8)],
        2: [(0, k) for k in range(8, 12)]
        + [(1, k) for k in range(8, 12)]
        + [(2, k) for k in range(8, 12)],
        3: [(0, k) for k in range(12, 16)]
        + [(1, k) for k in range(12, 16)]
        + [(2, k) for k in range(12, 16)],
    }

    with tile.TileContext(nc) as tc:
        with (
            tc.tile_pool(name="weights", bufs=1) as wpool,
            tc.tile_pool(name="xin", bufs=2) as xpool,
            tc.tile_pool(name="qkv", bufs=1) as qkvpool,
            tc.tile_pool(name="pt", bufs=1) as ptpool,
            tc.tile_pool(name="norm", bufs=1) as npool,
            tc.tile_pool(name="outsb", bufs=2) as opool,
        ):
            # ---- weights + constants -------------------------------------
            # weight DMAs ride the ACT HWDGE ring so they don't head-of-line
            # block the x stream on the SP ring
            wq_f = [wpool.tile([P, hd], F16, tag=f"wq{t}", name=f"wq{t}") for t in range(nf)]
            wk_f = [wpool.tile([P, hd], F16, tag=f"wk{t}", name=f"wk{t}") for t in range(nf)]
            wv_sb = wpool.tile([P, nf, hd], F16, tag="wv")
            wo_sb = wpool.tile([P, hd // P, o], F16, tag="wo")
            for ft in range(nf):
                nc.scalar.dma_start(wq_f[ft][:], wq_t[:, ft])
                nc.scalar.dma_start(wk_f[ft][:], wk_t[:, ft])
            nc.scalar.dma_start(wv_sb[:], wv_t)
            nc.scalar.dma_start(wo_sb[:], wo_t)
            # memset can't write fp16; memset fp32 scratch, cast-copy
            ones_f32 = wpool.tile([P, CH], F32, tag="ones_f32")
            nc.vector.memset(ones_f32[:], 1.0)
            ones_sb = wpool.tile([1, CH], F16, tag="ones")
            nc.vector.tensor_copy(ones_sb[:], ones_f32[0:1, :])

            # ---- storage -------------------------------------------------
            QT = [
                [qkvpool.tile([P, CH], F16, tag=f"QT{p_}_{c}", name=f"QT{p_}_{c}") for c in range(nch)]
                for p_ in range(npair)
            ]
            KT = [
                [qkvpool.tile([P, CH], F16, tag=f"KT{p_}_{c}", name=f"KT{p_}_{c}") for c in range(nch)]
                for p_ in range(npair)
            ]
            # V': per k-tile [128, hpc, DH+1]; last column is ones
            V = [qkvpool.tile([P, hpc, DH + 1], F16, tag=f"V{kt}", name=f"V{kt}") for kt in range(nkt)]
            YT = [
                [qkvpool.tile([P, CH], F16, tag=f"YT{p_}_{c}", name=f"YT{p_}_{c}") for c in range(nch)]
                for p_ in range(npair)
            ]
            for kt in range(nkt):
                nc.vector.tensor_copy(V[kt][:, :, DH], ones_f32[:, 0:hpc])

            blocks = [(c, p_) for c in range(nch) for p_ in range(npair)]

            # ps_att (scores PSUM) lives for the whole kernel: 4 banks.
            # Projection-phase pools add 4 more (within the 8-bank budget);
            # after they close, the psY pool takes 3 and ps_o 1.
            with tc.tile_pool(name="ps_att", bufs=2, space="PSUM") as ps_att:

                def emit_scores(p_, c, kt):
                    ps_s = ps_att.tile([P, 2 * CH], F32, tag="ps_s", name="ps_s")
                    nc.tensor.matmul(
                        ps_s[:, 0:CH],
                        KT[p_][kt // 4][0:DH, (kt % 4) * P : (kt % 4 + 1) * P],
                        QT[p_][c][0:DH, :],
                        tile_position=(0, 0),
                    )
                    nc.tensor.matmul(
                        ps_s[:, CH : 2 * CH],
                        KT[p_][kt // 4][DH : 2 * DH, (kt % 4) * P : (kt % 4 + 1) * P],
                        QT[p_][c][DH : 2 * DH, :],
                        tile_position=(DH, 0),
                    )
                    return ps_s

                # saved exp(scores) tiles; (block mod 3, kt) keys the SBUF
                # slot so 3 blocks can be in flight
                PT = {}

                def emit_score_exp(bi, kt):
                    c, p_ = blocks[bi]
                    ps_s = emit_scores(p_, c, kt)
                    pt = ptpool.tile(
                        [P, 2 * CH], F16, tag=f"pt{bi % 3}_{kt}", name=f"pt{bi % 3}_{kt}"
                    )
                    nc.scalar.activation(pt[:], ps_s[:], AF.Exp)
                    PT[(bi, kt)] = pt

                # ---- projections + 3-block score/exp prefill -------------
                # prefill units drip into the projection instruction stream
                # (each chunk's units flow out through later proj matmuls)
                # so the ACT engine is fed continuously, not in bursts
                prelist = []

                def drip(k):
                    for _ in range(min(k, len(prelist))):
                        bi, kt = prelist.pop(0)
                        emit_score_exp(bi, kt)

                with (
                    tc.tile_pool(name="ps_projqk", bufs=1, space="PSUM") as ps_projqk,
                    tc.tile_pool(name="ps_projv", bufs=2, space="PSUM") as ps_projv,
                ):

                    def dummy_mm():
                        # filler matmul whose only dependency is the first
                        # 64KB weight DMA (~0.3us): bridges DMA trickle gaps
                        # in the cold phase so HAM stays busy and releases
                        # the clock throttle early
                        ps_wu = ps_projv.tile([P, CH], F32, tag="psV", name="ps_wu")
                        nc.tensor.matmul(
                            ps_wu[0:32, 0:hd], wq_f[0][0:1, 0:32], wq_f[0][0:1, :]
                        )

                    # solid warm-up burst: ~4.5us of continuous PE activity
                    # takes HAM through a full SHORT window so the real
                    # projection matmuls run at 2.4GHz from the start
                    for wu in range(20):
                        dummy_mm()
                    for c in range(nch):
                        xq_t = xpool.tile([P, nf, CH], F16, tag="xq", name="xq_t")
                        xkv_t = xpool.tile([P, nf, CH], F16, tag="xkv", name="xkv_t")
                        if c <= 1:
                            # interleaved quarter-DMAs so both the Q and K
                            # passes can start on partial chunks instead of
                            # waiting for the full 2MB per stream
                            for qd in range(4):
                                ftsl = slice(2 * qd, 2 * qd + 2)
                                nc.sync.dma_start(
                                    xq_t[:, ftsl, :], xqT_t[:, ftsl, c * CH : (c + 1) * CH]
                                )
                                nc.sync.dma_start(
                                    xkv_t[:, ftsl, :], xkvT_t[:, ftsl, c * CH : (c + 1) * CH]
                                )
                        else:
                            nc.sync.dma_start(xq_t[:], xqT_t[:, :, c * CH : (c + 1) * CH])
                            nc.sync.dma_start(xkv_t[:], xkvT_t[:, :, c * CH : (c + 1) * CH])
                        # Q pass (K pass reuses the same PSUM tags)
                        psQ = [ps_projqk.tile([P, CH], F32, tag=f"psQK{m}", name="psQ") for m in range(npair)]
                        for ft in range(nf):
                            for m in range(npair):
                                nc.tensor.matmul(
                                    psQ[m][:],
                                    wq_f[ft][:, m * P : (m + 1) * P],
                                    xq_t[:, ft],
                                    start=(ft == 0),
                                    stop=(ft == nf - 1),
                                )
                            if ft % 2 == 1:
                                if prelist:
                                    drip(1)
                                elif c == 0:
                                    dummy_mm()
                        for m in range(npair):
                            nc.vector.tensor_copy(QT[m][c][:], psQ[m][:])
                        # K pass
                        psK = [ps_projqk.tile([P, CH], F32, tag=f"psQK{m}", name="psK") for m in range(npair)]
                        for ft in range(nf):
                            for m in range(npair):
                                nc.tensor.matmul(
                                    psK[m][:],
                                    wk_f[ft][:, m * P : (m + 1) * P],
                                    xkv_t[:, ft],
                                    start=(ft == 0),
                                    stop=(ft == nf - 1),
                                )
                            if ft % 2 == 1:
                                if prelist:
                                    drip(1)
                                elif c == 0:
                                    dummy_mm()
                        for m in range(npair):
                            nc.vector.tensor_copy(KT[m][c][:], psK[m][:])
                        prelist.extend(PREFILL[c])
                        drip(2)
                        # V pass (xkv chunk tile as lhsT); one PSUM
                        # accumulation group per bank, so st is outer
                        for st in range(4):
                            psV = ps_projv.tile([P, CH], F32, tag="psV", name="psV")
                            for ft in range(nf):
                                nc.tensor.matmul(
                                    psV[:, 0:hd],
                                    xkv_t[:, ft, st * P : (st + 1) * P],
                                    wv_sb[:, ft, :],
                                    start=(ft == 0),
                                    stop=(ft == nf - 1),
                                )
                            kt = c * 4 + st
                            nc.vector.tensor_copy(
                                V[kt][:, :, 0:DH],
                                psV[:, 0:hd].rearrange("p (h d) -> p h d", h=hpc),
                            )
                            drip(1)
                        drip(2)
                    # whatever prefill remains flows out before attention
                    drip(len(prelist))

                # deferred work queue: sub-microsecond units injected into
                # later kt iterations so no engine sees a burst
                pending = []
                ps_opool = []  # filled once the attention-phase PSUM pool opens

                def queue_normalize(p_, c, stages):
                    def emit(h01):
                        stage, den_r = stages[h01]
                        # broadcast the denominator row on the otherwise-idle
                        # GPSIMD engine (source must sit at partition 0)
                        bc_sb = npool.tile([DH, CH], F32, tag=f"bc{h01}", name="bc_sb")
                        nc.gpsimd.partition_broadcast(bc_sb[:], den_r[:])
                        inv_sb = npool.tile([DH, CH], F32, tag=f"inv{h01}", name="inv_sb")
                        nc.vector.reciprocal_approx_fast(out=inv_sb[:], in_=bc_sb[:])
                        nc.vector.tensor_tensor(
                            YT[p_][c][h01 * DH : (h01 + 1) * DH, :],
                            stage[:],
                            inv_sb[:],
                            mybir.AluOpType.mult,
                        )

                    pending.append(lambda: emit(0))
                    pending.append(lambda: emit(1))

                def queue_outproj(c):
                    for st in range(4):
                        qt = c * 4 + st
                        carrier = {}

                        def emit_half(j, st=st, c=c, carrier=carrier):
                            if j == 0:
                                carrier["out_sb"] = opool.tile([P, o], F16, tag="out_sb", name="out_sb")
                            ps_o = ps_opool[0].tile([P, CH], F32, tag="ps_o", name="ps_o")
                            for m in range(hd // P):
                                nc.tensor.matmul(
                                    ps_o[:],
                                    YT[m][c][:, st * P : (st + 1) * P],
                                    wo_sb[:, m, j * CH : (j + 1) * CH],
                                    start=(m == 0),
                                    stop=(m == hd // P - 1),
                                )
                            nc.vector.tensor_copy(
                                carrier["out_sb"][:, j * CH : (j + 1) * CH], ps_o[:]
                            )

                        def emit_dma(qt=qt, carrier=carrier):
                            nc.sync.dma_start(
                                out.ap()[qt * P : (qt + 1) * P, :], carrier["out_sb"][:]
                            )

                        pending.append(lambda f_=emit_half: f_(0))
                        pending.append(lambda f_=emit_half: f_(1))
                        pending.append(emit_dma)

                # ---- attention blocks ------------------------------------
                # blocks 0-2 fully prefilled; during block bi, emit
                # (bi+3, kt) right after this unit's AVs free the pt slot
                with (
                    tc.tile_pool(name="ps_y", bufs=3, space="PSUM") as ps_ypool,
                    tc.tile_pool(name="ps_o", bufs=1, space="PSUM") as ps_opool_,
                ):
                    ps_opool.append(ps_opool_)
                    for bi, (c, p_) in enumerate(blocks):
                        hA, hB = 2 * p_, 2 * p_ + 1
                        psY = [
                            ps_ypool.tile([DH + 1, CH], F32, tag="psY", name="psY_a"),
                            ps_ypool.tile([DH + 1, CH], F32, tag="psY", name="psY_b"),
                        ]
                        for kt in range(nkt):
                            pt = PT.pop((bi, kt))
                            nc.tensor.matmul(
                                psY[0][:],
                                V[kt][:, hA, :],
                                pt[:, 0:CH],
                                start=(kt == 0),
                                stop=(kt == nkt - 1),
                            )
                            nc.tensor.matmul(
                                psY[1][:],
                                V[kt][:, hB, :],
                                pt[:, CH : 2 * CH],
                                start=(kt == 0),
                                stop=(kt == nkt - 1),
                            )
                            if pending:
                                pending.pop(0)()
                            if bi + 3 < nblk:
                                emit_score_exp(bi + 3, kt)
                        # stage psY out to SBUF immediately so the PSUM banks
                        # free up for the next block's AV matmuls
                        stages = []
                        for h01 in range(2):
                            stage = npool.tile(
                                [DH, CH], F32, tag=f"stage{h01}", name="stage"
                            )
                            nc.vector.tensor_copy(stage[:], psY[h01][0:DH, :])
                            den_r = npool.tile([1, CH], F32, tag=f"den{h01}", name="den_r")
                            nc.vector.tensor_copy(den_r[:], psY[h01][DH : DH + 1, :])
                            stages.append((stage, den_r))
                        queue_normalize(p_, c, stages)
                        if p_ == npair - 1:
                            queue_outproj(c)
                    while pending:
                        pending.pop(0)()

    nc.compile()
    return nc


def make_in_maps(inputs_q, inputs_kv, wq, wk, wv, wo):
    """Shard full inputs into 8 per-core input dicts (host-side)."""
    in_maps = []
    scale = 1.0 / np.sqrt(DH)
    for core in range(NCORES):
        b = core // (NCORES // B)
        hg = core % (NCORES // B)
        hs = slice(hg * HPC, (hg + 1) * HPC)
        in_maps.append(
            {
                "xqT": np.ascontiguousarray(inputs_q[b].T).astype(np.float16),
                "xkvT": np.ascontiguousarray(inputs_kv[b].T).astype(np.float16),
                "wq": np.ascontiguousarray(
                    (wq[:, hs, :] * scale).reshape(F, HPC * DH)
                ).astype(np.float16),
                "wk": np.ascontiguousarray(wk[:, hs, :].reshape(F, HPC * DH)).astype(
                    np.float16
                ),
                "wv": np.ascontiguousarray(wv[:, hs, :].reshape(F, HPC * DH)).astype(
                    np.float16
                ),
                "wo": np.ascontiguousarray(wo[hs].reshape(HPC * DH, O)).astype(
                    np.float16
                ),
            }
        )
    return in_maps


_CACHE = {}


def _get_program():
    if "nc" not in _CACHE:
        _CACHE["nc"] = build_program()
    return _CACHE["nc"]


def run_sharded(inputs_q, inputs_kv, wq, wk, wv, wo, bo, **spmd_kwargs):
    """Build in_maps, run on 8 cores, reduce partials. Returns (out, results)."""
    nc = _get_program()
    in_maps = make_in_maps(inputs_q, inputs_kv, wq, wk, wv, wo)
    res = run_bass_kernel_spmd(nc, in_maps, core_ids=list(range(NCORES)), **spmd_kwargs)
    gpb = NCORES // B  # head-group cores per batch element
    out = np.zeros((B, S, O), dtype=np.float32)
    for core in range(NCORES):
        out[core // gpb] += res.results[core]["out"].astype(np.float32)
    out += np.asarray(bo, dtype=np.float32)
    return out, res


def kernel(inputs_q, inputs_kv, wq, wk, wv, wo, bo):
    out, _ = run_sharded(
        np.asarray(inputs_q),
        np.asarray(inputs_kv),
        np.asarray(wq),
        np.asarray(wk),
        np.asarray(wv),
        np.asarray(wo),
        np.asarray(bo),
    )
    return out


# revision 18
# speedup vs baseline: 1.0005x; 1.0005x over previous
"""Multi-head dot-product attention (B=2, S=2048, F=1024, H=16, DH=64, O=1024)
as a Bass/Tile kernel on 8 Trainium2 NeuronCores.

Sharding: data-parallel over B (2) x tensor-parallel over H (4 groups of 4
heads) = 8 cores. Each core computes q/k/v projections for its 4 heads,
softmax attention, and a partial output projection; the host sums the 4
partial outputs per batch element and adds the bias.

Device layouts (per core):
  xqT, xkvT  [F, S]  fp16   host-pre-transposed activations
  wq, wk, wv [F, 4*DH] fp16 weight shards (wq pre-scaled by 1/sqrt(DH))
  wo         [4*DH, O] fp16
  out        [S, O]  fp16   partial output (host accumulates in fp32)

Attention works in transposed-score space: sT[k, q] = KT_slice.T @ QT (two
heads packed into PE row-groups 0-63 / 64-127), one exp on ACT covers both
heads, then y'T = V'.T @ PT where V' carries a ones column so row 64 of y'T
accumulates the softmax denominator (scores are O(1), so max-subtraction is
unnecessary).

Pipeline (v2): the exp stream on ACT (135us total) and the matmul stream on
PE (~150us warm) are the two poles; the schedule keeps both dense:
  - chunk 0's x DMAs are split in quarters so real projection matmuls start
    ~1us into the kernel and HAM never re-throttles;
  - during the projection phase the first 3 blocks' scores+exp prefill
    (48/128 units) on the otherwise-idle ACT engine (pt tiles rotate 3-deep
    so 3 blocks can be in flight);
  - in the attention phase each (block, kt) unit runs 2 AV matmuls, pops one
    deferred normalization/out-projection item, and emits scores+exp for
    (block+3, kt) into the pt slot its own AV just freed;
  - psY PSUM tiles rotate through 3 banks and are staged out to SBUF right
    at block end, so the next block's AV matmuls never wait on PSUM;
  - the output rides fp16 DMA (host accumulates partial sums in fp32).
"""

import numpy as np

import concourse.bass as bass
import concourse.mybir as mybir
import concourse.tile as tile
from concourse import bacc
from concourse.bass_utils import run_bass_kernel_spmd

F32 = mybir.dt.float32
F16 = mybir.dt.float16
AF = mybir.ActivationFunctionType

B, S, F, H, DH, O = 2, 2048, 1024, 16, 64, 1024
NCORES = 8
HPC = 4  # heads per core
CH = 512  # q-chunk width
P = 128


def build_program(s=S, f=F, o=O, hpc=HPC):
    npair = hpc // 2
    nch = s // CH  # q chunks
    nkt = s // P  # k tiles
    nf = f // P  # contraction tiles for projections
    hd = hpc * DH  # stacked head dims per core (256)
    nblk = nch * npair

    nc = bacc.Bacc("TRN2", target_bir_lowering=False, debug=False, num_devices=NCORES)

    xqT = nc.dram_tensor("xqT", [f, s], F16, kind="ExternalInput")
    xkvT = nc.dram_tensor("xkvT", [f, s], F16, kind="ExternalInput")
    wq = nc.dram_tensor("wq", [f, hd], F16, kind="ExternalInput")
    wk = nc.dram_tensor("wk", [f, hd], F16, kind="ExternalInput")
    wv = nc.dram_tensor("wv", [f, hd], F16, kind="ExternalInput")
    wo = nc.dram_tensor("wo", [hd, o], F16, kind="ExternalInput")
    out = nc.dram_tensor("out", [s, o], F16, kind="ExternalOutput")

    xqT_t = xqT.ap().rearrange("(t p) n -> p t n", p=P)  # [128, nf, s]
    xkvT_t = xkvT.ap().rearrange("(t p) n -> p t n", p=P)
    wq_t = wq.ap().rearrange("(t p) n -> p t n", p=P)  # [128, nf, hd]
    wk_t = wk.ap().rearrange("(t p) n -> p t n", p=P)
    wv_t = wv.ap().rearrange("(t p) n -> p t n", p=P)
    wo_t = wo.ap().rearrange("(t p) n -> p t n", p=P)  # [128, hd//128, o]

    # per-chunk prefill schedule: after chunk c's Q/K/V projections, these
    # (block, kt) units' scores+exp are emitted (feasible: block's q-chunk
    # and kt//4 are both <= c; capacity: 3 pt parities)
    PREFILL = {
        0: [(0, k) for k in range(4)] + [(1, k) for k in range(4)],
        1: [(0, k) for k in range(4, 8)]
        + [(1, k) for k in range(4, 8)]
        + [(2, k) for k in range(8)],
        2: [(0, k) for k in range(8, 12)]
        + [(1, k) for k in range(8, 12)]
        + [(2, k) for k in range(8, 12)],
        3: [(0, k) for k in range(12, 16)]
        + [(1, k) for k in range(12, 16)]
        + [(2, k) for k in range(12, 16)],
    }

    with tile.TileContext(nc) as tc:
        with (
            tc.tile_pool(name="weights", bufs=1) as wpool,
            tc.tile_pool(name="xin", bufs=2) as xpool,
            tc.tile_pool(name="qkv", bufs=1) as qkvpool,
            tc.tile_pool(name="pt", bufs=1) as ptpool,
            tc.tile_pool(name="norm", bufs=1) as npool,
            tc.tile_pool(name="outsb", bufs=2) as opool,
        ):
            # ---- weights + constants -------------------------------------
            # weight DMAs ride the ACT HWDGE ring so they don't head-of-line
            # block the x stream on the SP ring
            wq_f = [wpool.tile([P, hd], F16, tag=f"wq{t}", name=f"wq{t}") for t in range(nf)]
            wk_f = [wpool.tile([P, hd], F16, tag=f"wk{t}", name=f"wk{t}") for t in range(nf)]
            wv_sb = wpool.tile([P, nf, hd], F16, tag="wv")
            wo_sb = wpool.tile([P, hd // P, o], F16, tag="wo")
            for ft in range(nf):
                nc.scalar.dma_start(wq_f[ft][:], wq_t[:, ft])
                nc.scalar.dma_start(wk_f[ft][:], wk_t[:, ft])
            # wv/wo DMAs are deferred into the chunk-0 loop so they don't
            # steal HBM bandwidth from the critical first x chunk
            # memset can't write fp16; memset fp32 scratch, cast-copy
            ones_f32 = wpool.tile([P, CH], F32, tag="ones_f32")
            nc.vector.memset(ones_f32[:], 1.0)
            ones_sb = wpool.tile([1, CH], F16, tag="ones")
            nc.vector.tensor_copy(ones_sb[:], ones_f32[0:1, :])

            # ---- storage -------------------------------------------------
            QT = [
                [qkvpool.tile([P, CH], F16, tag=f"QT{p_}_{c}", name=f"QT{p_}_{c}") for c in range(nch)]
                for p_ in range(npair)
            ]
            KT = [
                [qkvpool.tile([P, CH], F16, tag=f"KT{p_}_{c}", name=f"KT{p_}_{c}") for c in range(nch)]
                for p_ in range(npair)
            ]
            # V': per k-tile [128, hpc, DH+1]; last column is ones
            V = [qkvpool.tile([P, hpc, DH + 1], F16, tag=f"V{kt}", name=f"V{kt}") for kt in range(nkt)]
            YT = [
                [qkvpool.tile([P, CH], F16, tag=f"YT{p_}_{c}", name=f"YT{p_}_{c}") for c in range(nch)]
                for p_ in range(npair)
            ]
            for kt in range(nkt):
                nc.vector.tensor_copy(V[kt][:, :, DH], ones_f32[:, 0:hpc])

            blocks = [(c, p_) for c in range(nch) for p_ in range(npair)]

            # ps_att (scores PSUM) lives for the whole kernel: 4 banks.
            # Projection-phase pools add 4 more (within the 8-bank budget);
            # after they close, the psY pool takes 3 and ps_o 1.
            with tc.tile_pool(name="ps_att", bufs=2, space="PSUM") as ps_att:

                def emit_scores(p_, c, kt):
                    ps_s = ps_att.tile([P, 2 * CH], F32, tag="ps_s", name="ps_s")
                    nc.tensor.matmul(
                        ps_s[:, 0:CH],
                        KT[p_][kt // 4][0:DH, (kt % 4) * P : (kt % 4 + 1) * P],
                        QT[p_][c][0:DH, :],
                        tile_position=(0, 0),
                    )
                    nc.tensor.matmul(
                        ps_s[:, CH : 2 * CH],
                        KT[p_][kt // 4][DH : 2 * DH, (kt % 4) * P : (kt % 4 + 1) * P],
                        QT[p_][c][DH : 2 * DH, :],
                        tile_position=(DH, 0),
                    )
                    return ps_s

                # saved exp(scores) tiles; (block mod 3, kt) keys the SBUF
                # slot so 3 blocks can be in flight
                PT = {}

                def emit_score_exp(bi, kt):
                    c, p_ = blocks[bi]
                    ps_s = emit_scores(p_, c, kt)
                    pt = ptpool.tile(
                        [P, 2 * CH], F16, tag=f"pt{bi % 3}_{kt}", name=f"pt{bi % 3}_{kt}"
                    )
                    nc.scalar.activation(pt[:], ps_s[:], AF.Exp)
                    PT[(bi, kt)] = pt

                # ---- projections + 3-block score/exp prefill -------------
                # prefill units drip into the projection instruction stream
                # (each chunk's units flow out through later proj matmuls)
                # so the ACT engine is fed continuously, not in bursts
                prelist = []

                def drip(k):
                    for _ in range(min(k, len(prelist))):
                        bi, kt = prelist.pop(0)
                        emit_score_exp(bi, kt)

                with (
                    tc.tile_pool(name="ps_projqk", bufs=1, space="PSUM") as ps_projqk,
                    tc.tile_pool(name="ps_projv", bufs=2, space="PSUM") as ps_projv,
                ):

                    def dummy_mm():
                        # filler matmul whose only dependency is the first
                        # 64KB weight DMA: bridges DMA trickle gaps in the
                        # cold phase. Full K=128 so the array lights up and
                        # HAM sees real activity (K=1 fillers don't count)
                        # and releases the clock throttle early.
                        ps_wu = ps_projv.tile([P, CH], F32, tag="psV", name="ps_wu")
                        nc.tensor.matmul(
                            ps_wu[:, 0:hd], wq_f[0][:, 0:P], wq_f[0][:, :]
                        )

                    # solid warm-up burst: ~3.5us of continuous PE activity
                    # takes HAM through a full SHORT window so the real
                    # projection matmuls run at 2.4GHz as soon as possible
                    for wu in range(16):
                        dummy_mm()
                    xtiles = {}
                    for c in range(nch):
                        xtiles[c] = (
                            xpool.tile([P, nf, CH], F16, tag="xq", name="xq_t"),
                            xpool.tile([P, nf, CH], F16, tag="xkv", name="xkv_t"),
                        )

                    def issue_x_dma(c):
                        xq_t, xkv_t = xtiles[c]
                        if c == 0:
                            # interleaved quarter-DMAs so both the Q and K
                            # passes can start on partial chunks instead of
                            # waiting for the full 2MB per stream
                            for qd in range(4):
                                ftsl = slice(2 * qd, 2 * qd + 2)
                                nc.sync.dma_start(
                                    xq_t[:, ftsl, :], xqT_t[:, ftsl, c * CH : (c + 1) * CH]
                                )
                                nc.sync.dma_start(
                                    xkv_t[:, ftsl, :], xkvT_t[:, ftsl, c * CH : (c + 1) * CH]
                                )
                        else:
                            nc.sync.dma_start(xq_t[:], xqT_t[:, :, c * CH : (c + 1) * CH])
                            nc.sync.dma_start(xkv_t[:], xkvT_t[:, :, c * CH : (c + 1) * CH])

                    issue_x_dma(0)
                    for c in range(nch):
                        xq_t, xkv_t = xtiles[c]
                        # Q pass (K pass reuses the same PSUM tags)
                        psQ = [ps_projqk.tile([P, CH], F32, tag=f"psQK{m}", name="psQ") for m in range(npair)]
                        for ft in range(nf):
                            for m in range(npair):
                                nc.tensor.matmul(
                                    psQ[m][:],
                                    wq_f[ft][:, m * P : (m + 1) * P],
                                    xq_t[:, ft],
                                    start=(ft == 0),
                                    stop=(ft == nf - 1),
                                )
                            if ft % 2 == 1:
                                if prelist:
                                    drip(1)
                                elif c == 0:
                                    dummy_mm()
                        for m in range(npair):
                            nc.vector.tensor_copy(QT[m][c][:], psQ[m][:])
                        if c == 0:
                            # deferred weight DMAs: issued only now so they
                            # don't compete with chunk 0's x stream
                            nc.scalar.dma_start(wv_sb[:], wv_t)
                            nc.scalar.dma_start(wo_sb[:], wo_t)
                        elif c + 1 < nch:
                            issue_x_dma(c + 1)
                        # K pass
                        psK = [ps_projqk.tile([P, CH], F32, tag=f"psQK{m}", name="psK") for m in range(npair)]
                        for ft in range(nf):
                            for m in range(npair):
                                nc.tensor.matmul(
                                    psK[m][:],
                                    wk_f[ft][:, m * P : (m + 1) * P],
                                    xkv_t[:, ft],
                                    start=(ft == 0),
                                    stop=(ft == nf - 1),
                                )
                            if ft % 2 == 1:
                                if prelist:
                                    drip(1)
                                elif c == 0:
                                    dummy_mm()
                        for m in range(npair):
                            nc.vector.tensor_copy(KT[m][c][:], psK[m][:])
                        if c == 0:
                            issue_x_dma(1)
                        prelist.extend(PREFILL[c])
                        drip(2)
                        # V pass (xkv chunk tile as lhsT); one PSUM
                        # accumulation group per bank, so st is outer
                        for st in range(4):
                            psV = ps_projv.tile([P, CH], F32, tag="psV", name="psV")
                            for ft in range(nf):
                                nc.tensor.matmul(
                                    psV[:, 0:hd],
                                    xkv_t[:, ft, st * P : (st + 1) * P],
                                    wv_sb[:, ft, :],
                                    start=(ft == 0),
                                    stop=(ft == nf - 1),
                                )
                            kt = c * 4 + st
                            nc.vector.tensor_copy(
                                V[kt][:, :, 0:DH],
                                psV[:, 0:hd].rearrange("p (h d) -> p h d", h=hpc),
                            )
                            drip(1)
                        drip(2)
                    # whatever prefill remains flows out before attention
                    drip(len(prelist))

                # deferred work queue: sub-microsecond units injected into
                # later kt iterations so no engine sees a burst
                pending = []
                ps_opool = []  # filled once the attention-phase PSUM pool opens

                def queue_normalize(p_, c, stages):
                    def emit(h01):
                        stage, den_r = stages[h01]
                        # broadcast the denominator row on the otherwise-idle
                        # GPSIMD engine (source must sit at partition 0)
                        bc_sb = npool.tile([DH, CH], F32, tag=f"bc{h01}", name="bc_sb")
                        nc.gpsimd.partition_broadcast(bc_sb[:], den_r[:])
                        inv_sb = npool.tile([DH, CH], F32, tag=f"inv{h01}", name="inv_sb")
                        nc.vector.reciprocal_approx_fast(out=inv_sb[:], in_=bc_sb[:])
                        nc.vector.tensor_tensor(
                            YT[p_][c][h01 * DH : (h01 + 1) * DH, :],
                            stage[:],
                            inv_sb[:],
                            mybir.AluOpType.mult,
                        )

                    pending.append(lambda: emit(0))
                    pending.append(lambda: emit(1))

                def queue_outproj(c):
                    for st in range(4):
                        qt = c * 4 + st
                        carrier = {}

                        def emit_half(j, st=st, c=c, carrier=carrier):
                            if j == 0:
                                carrier["out_sb"] = opool.tile([P, o], F16, tag="out_sb", name="out_sb")
                            ps_o = ps_opool[0].tile([P, CH], F32, tag="ps_o", name="ps_o")
                            for m in range(hd // P):
                                nc.tensor.matmul(
                                    ps_o[:],
                                    YT[m][c][:, st * P : (st + 1) * P],
                                    wo_sb[:, m, j * CH : (j + 1) * CH],
                                    start=(m == 0),
                                    stop=(m == hd // P - 1),
                                )
                            nc.vector.tensor_copy(
                                carrier["out_sb"][:, j * CH : (j + 1) * CH], ps_o[:]
                            )

                        def emit_dma(qt=qt, carrier=carrier):
                            nc.sync.dma_start(
                                out.ap()[qt * P : (qt + 1) * P, :], carrier["out_sb"][:]
                            )

                        pending.append(lambda f_=emit_half: f_(0))
                        pending.append(lambda f_=emit_half: f_(1))
                        pending.append(emit_dma)

                # ---- attention blocks ------------------------------------
                # blocks 0-2 fully prefilled; during block bi, emit
                # (bi+3, kt) right after this unit's AVs free the pt slot
                with (
                    tc.tile_pool(name="ps_y", bufs=3, space="PSUM") as ps_ypool,
                    tc.tile_pool(name="ps_o", bufs=1, space="PSUM") as ps_opool_,
                ):
                    ps_opool.append(ps_opool_)
                    for bi, (c, p_) in enumerate(blocks):
                        hA, hB = 2 * p_, 2 * p_ + 1
                        psY = [
                            ps_ypool.tile([DH + 1, CH], F32, tag="psY", name="psY_a"),
                            ps_ypool.tile([DH + 1, CH], F32, tag="psY", name="psY_b"),
                        ]
                        for kt in range(nkt):
                            pt = PT.pop((bi, kt))
                            nc.tensor.matmul(
                                psY[0][:],
                                V[kt][:, hA, :],
                                pt[:, 0:CH],
                                start=(kt == 0),
                                stop=(kt == nkt - 1),
                            )
                            nc.tensor.matmul(
                                psY[1][:],
                                V[kt][:, hB, :],
                                pt[:, CH : 2 * CH],
                                start=(kt == 0),
                                stop=(kt == nkt - 1),
                            )
                            if pending:
                                pending.pop(0)()
                            if bi + 3 < nblk:
                                emit_score_exp(bi + 3, kt)
                        # stage psY out to SBUF immediately so the PSUM banks
                        # free up for the next block's AV matmuls
                        stages = []
                        for h01 in range(2):
                            stage = npool.tile(
                                [DH, CH], F32, tag=f"stage{h01}", name="stage"
                            )
                            nc.vector.tensor_copy(stage[:], psY[h01][0:DH, :])
                            den_r = npool.tile([1, CH], F32, tag=f"den{h01}", name="den_r")
                            nc.vector.tensor_copy(den_r[:], psY[h01][DH : DH + 1, :])
                            stages.append((stage, den_r))
                        queue_normalize(p_, c, stages)
                        if p_ == npair - 1:
                            queue_outproj(c)
                    while pending:
                        pending.pop(0)()

    nc.compile()
    return nc


def make_in_maps(inputs_q, inputs_kv, wq, wk, wv, wo):
    """Shard full inputs into 8 per-core input dicts (host-side)."""
    in_maps = []
    scale = 1.0 / np.sqrt(DH)
    for core in range(NCORES):
        b = core // (NCORES // B)
        hg = core % (NCORES // B)
        hs = slice(hg * HPC, (hg + 1) * HPC)
        in_maps.append(
            {
                "xqT": np.ascontiguousarray(inputs_q[b].T).astype(np.float16),
                "xkvT": np.ascontiguousarray(inputs_kv[b].T).astype(np.float16),
                "wq": np.ascontiguousarray(
                    (wq[:, hs, :] * scale).reshape(F, HPC * DH)
                ).astype(np.float16),
                "wk": np.ascontiguousarray(wk[:, hs, :].reshape(F, HPC * DH)).astype(
                    np.float16
                ),
                "wv": np.ascontiguousarray(wv[:, hs, :].reshape(F, HPC * DH)).astype(
                    np.float16
                ),
                "wo": np.ascontiguousarray(wo[hs].reshape(HPC * DH, O)).astype(
                    np.float16
                ),
            }
        )
    return in_maps


_CACHE = {}


def _get_program():
    if "nc" not in _CACHE:
        _CACHE["nc"] = build_program()
    return _CACHE["nc"]


def run_sharded(inputs_q, inputs_kv, wq, wk, wv, wo, bo, **spmd_kwargs):
    """Build in_maps, run on 8 cores, reduce partials. Returns (out, results)."""
    nc = _get_program()
    in_maps = make_in_maps(inputs_q, inputs_kv, wq, wk, wv, wo)
    res = run_bass_kernel_spmd(nc, in_maps, core_ids=list(range(NCORES)), **spmd_kwargs)
    gpb = NCORES // B  # head-group cores per batch element
    out = np.zeros((B, S, O), dtype=np.float32)
    for core in range(NCORES):
        out[core // gpb] += res.results[core]["out"].astype(np.float32)
    out += np.asarray(bo, dtype=np.float32)
    return out, res


def kernel(inputs_q, inputs_kv, wq, wk, wv, wo, bo):
    out, _ = run_sharded(
        np.asarray(inputs_q),
        np.asarray(inputs_kv),
        np.asarray(wq),
        np.asarray(wk),
        np.asarray(wv),
        np.asarray(wo),
        np.asarray(bo),
    )
    return out


# revision 26
# speedup vs baseline: 1.0188x; 1.0183x over previous
"""Multi-head dot-product attention (B=2, S=2048, F=1024, H=16, DH=64, O=1024)
as a Bass/Tile kernel on 8 Trainium2 NeuronCores.

Sharding: data-parallel over B (2) x tensor-parallel over H (4 groups of 4
heads) = 8 cores. Each core computes q/k/v projections for its 4 heads,
softmax attention, and a partial output projection; the host sums the 4
partial outputs per batch element and adds the bias.

Device layouts (per core):
  xqT, xkvT  [F, S]  fp16   host-pre-transposed activations
  wq, wk, wv [F, 4*DH] fp16 weight shards (wq pre-scaled by 1/sqrt(DH))
  wo         [4*DH, O] fp16
  out        [S, O]  fp16   partial output (host accumulates in fp32)

Attention works in transposed-score space: sT[k, q] = KT_slice.T @ QT (two
heads packed into PE row-groups 0-63 / 64-127), one exp on ACT covers both
heads, then y'T = V'.T @ PT where V' carries a ones column so row 64 of y'T
accumulates the softmax denominator (scores are O(1), so max-subtraction is
unnecessary).

Pipeline (v2): the exp stream on ACT (135us total) and the matmul stream on
PE (~150us warm) are the two poles; the schedule keeps both dense:
  - chunk 0's x DMAs are split in quarters so real projection matmuls start
    ~1us into the kernel and HAM never re-throttles;
  - during the projection phase the first 3 blocks' scores+exp prefill
    (48/128 units) on the otherwise-idle ACT engine (pt tiles rotate 3-deep
    so 3 blocks can be in flight);
  - in the attention phase each (block, kt) unit runs 2 AV matmuls, pops one
    deferred normalization/out-projection item, and emits scores+exp for
    (block+3, kt) into the pt slot its own AV just freed;
  - psY PSUM tiles rotate through 3 banks and are staged out to SBUF right
    at block end, so the next block's AV matmuls never wait on PSUM;
  - the output rides fp16 DMA (host accumulates partial sums in fp32).
"""

import numpy as np

import concourse.bass as bass
import concourse.mybir as mybir
import concourse.tile as tile
from concourse import bacc
from concourse.bass_utils import run_bass_kernel_spmd

F32 = mybir.dt.float32
F16 = mybir.dt.float16
AF = mybir.ActivationFunctionType

B, S, F, H, DH, O = 2, 2048, 1024, 16, 64, 1024
NCORES = 8
HPC = 4  # heads per core
CH = 512  # q-chunk width
P = 128


def build_program(s=S, f=F, o=O, hpc=HPC):
    npair = hpc // 2
    nch = s // CH  # q chunks
    nkt = s // P  # k tiles
    nf = f // P  # contraction tiles for projections
    hd = hpc * DH  # stacked head dims per core (256)
    nblk = nch * npair

    nc = bacc.Bacc("TRN2", target_bir_lowering=False, debug=False, num_devices=NCORES)

    xqT = nc.dram_tensor("xqT", [f, s], F16, kind="ExternalInput")
    xkvT = nc.dram_tensor("xkvT", [f, s], F16, kind="ExternalInput")
    wq = nc.dram_tensor("wq", [f, hd], F16, kind="ExternalInput")
    wk = nc.dram_tensor("wk", [f, hd], F16, kind="ExternalInput")
    wv = nc.dram_tensor("wv", [f, hd], F16, kind="ExternalInput")
    wo = nc.dram_tensor("wo", [hd, o], F16, kind="ExternalInput")
    out = nc.dram_tensor("out", [s, o], F16, kind="ExternalOutput")

    xqT_t = xqT.ap().rearrange("(t p) n -> p t n", p=P)  # [128, nf, s]
    xkvT_t = xkvT.ap().rearrange("(t p) n -> p t n", p=P)
    wq_t = wq.ap().rearrange("(t p) n -> p t n", p=P)  # [128, nf, hd]
    wk_t = wk.ap().rearrange("(t p) n -> p t n", p=P)
    wv_t = wv.ap().rearrange("(t p) n -> p t n", p=P)
    wo_t = wo.ap().rearrange("(t p) n -> p t n", p=P)  # [128, hd//128, o]

    # per-chunk prefill schedule: after chunk c's Q/K/V projections, these
    # (block, kt) units' scores+exp are emitted (feasible: block's q-chunk
    # and kt//4 are both <= c; capacity: 3 pt parities)
    PREFILL = {
        0: [(0, k) for k in range(4)] + [(1, k) for k in range(4)],
        1: [(0, k) for k in range(4, 8)]
        + [(1, k) for k in range(4, 8)]
        + [(2, k) for k in range(8)],
        2: [(0, k) for k in range(8, 12)]
        + [(1, k) for k in range(8, 12)]
        + [(2, k) for k in range(8, 12)],
        3: [(0, k) for k in range(12, 16)]
        + [(1, k) for k in range(12, 16)]
        + [(2, k) for k in range(12, 16)],
    }

    with tile.TileContext(nc) as tc:
        with (
            tc.tile_pool(name="weights", bufs=1) as wpool,
            tc.tile_pool(name="xin", bufs=2) as xpool,
            tc.tile_pool(name="qkv", bufs=1) as qkvpool,
            tc.tile_pool(name="pt", bufs=1) as ptpool,
            tc.tile_pool(name="norm", bufs=1) as npool,
            tc.tile_pool(name="outsb", bufs=2) as opool,
        ):
            # ---- weights + constants -------------------------------------
            # weight DMAs ride the ACT HWDGE ring so they don't head-of-line
            # block the x stream on the SP ring
            wq_f = [wpool.tile([P, hd], F16, tag=f"wq{t}", name=f"wq{t}") for t in range(nf)]
            wk_f = [wpool.tile([P, hd], F16, tag=f"wk{t}", name=f"wk{t}") for t in range(nf)]
            wv_sb = wpool.tile([P, nf, hd], F16, tag="wv")
            wo_sb = wpool.tile([P, hd // P, o], F16, tag="wo")
            for ft in range(nf):
                nc.scalar.dma_start(wq_f[ft][:], wq_t[:, ft])
                nc.scalar.dma_start(wk_f[ft][:], wk_t[:, ft])
            nc.scalar.dma_start(wv_sb[:], wv_t)
            nc.scalar.dma_start(wo_sb[:], wo_t)
            # memset can't write fp16; memset fp32 scratch, cast-copy
            ones_f32 = wpool.tile([P, CH], F32, tag="ones_f32")
            nc.vector.memset(ones_f32[:], 1.0)
            ones_sb = wpool.tile([1, CH], F16, tag="ones")
            nc.vector.tensor_copy(ones_sb[:], ones_f32[0:1, :])

            # ---- storage -------------------------------------------------
            QT = [
                [qkvpool.tile([P, CH], F16, tag=f"QT{p_}_{c}", name=f"QT{p_}_{c}") for c in range(nch)]
                for p_ in range(npair)
            ]
            KT = [
                [qkvpool.tile([P, CH], F16, tag=f"KT{p_}_{c}", name=f"KT{p_}_{c}") for c in range(nch)]
                for p_ in range(npair)
            ]
            # V': per k-tile [128, hpc, DH+1]; last column is ones
            V = [qkvpool.tile([P, hpc, DH + 1], F16, tag=f"V{kt}", name=f"V{kt}") for kt in range(nkt)]
            YT = [
                [qkvpool.tile([P, CH], F16, tag=f"YT{p_}_{c}", name=f"YT{p_}_{c}") for c in range(nch)]
                for p_ in range(npair)
            ]
            for kt in range(nkt):
                nc.vector.tensor_copy(V[kt][:, :, DH], ones_f32[:, 0:hpc])

            blocks = [(c, p_) for c in range(nch) for p_ in range(npair)]

            # ps_att (scores PSUM) lives for the whole kernel: 4 banks.
            # Projection-phase pools add 4 more (within the 8-bank budget);
            # after they close, the psY pool takes 3 and ps_o 1.
            with tc.tile_pool(name="ps_att", bufs=2, space="PSUM") as ps_att:

                def emit_scores(p_, c, kt):
                    ps_s = ps_att.tile([P, 2 * CH], F32, tag="ps_s", name="ps_s")
                    nc.tensor.matmul(
                        ps_s[:, 0:CH],
                        KT[p_][kt // 4][0:DH, (kt % 4) * P : (kt % 4 + 1) * P],
                        QT[p_][c][0:DH, :],
                        tile_position=(0, 0),
                    )
                    nc.tensor.matmul(
                        ps_s[:, CH : 2 * CH],
                        KT[p_][kt // 4][DH : 2 * DH, (kt % 4) * P : (kt % 4 + 1) * P],
                        QT[p_][c][DH : 2 * DH, :],
                        tile_position=(DH, 0),
                    )
                    return ps_s

                # saved exp(scores) tiles; (block mod 3, kt) keys the SBUF
                # slot so 3 blocks can be in flight
                PT = {}

                def emit_score_exp(bi, kt):
                    # high priority: the scheduler places scores+exp as early
                    # as dependencies allow, keeping the ACT pipeline fed
                    with tc.high_priority():
                        c, p_ = blocks[bi]
                        ps_s = emit_scores(p_, c, kt)
                        pt = ptpool.tile(
                            [P, 2 * CH], F16, tag=f"pt{bi % 3}_{kt}", name=f"pt{bi % 3}_{kt}"
                        )
                        nc.scalar.activation(pt[:], ps_s[:], AF.Exp)
                        PT[(bi, kt)] = pt

                # ---- projections + 3-block score/exp prefill -------------
                # prefill units drip into the projection instruction stream
                # (each chunk's units flow out through later proj matmuls)
                # so the ACT engine is fed continuously, not in bursts
                prelist = []

                def drip(k):
                    for _ in range(min(k, len(prelist))):
                        bi, kt = prelist.pop(0)
                        emit_score_exp(bi, kt)

                with (
                    tc.tile_pool(name="ps_projqk", bufs=1, space="PSUM") as ps_projqk,
                    tc.tile_pool(name="ps_projv", bufs=2, space="PSUM") as ps_projv,
                ):

                    def dummy_mm():
                        # filler matmul whose only dependency is the first
                        # 64KB weight DMA: bridges DMA trickle gaps in the
                        # cold phase. Full K=128 so the array lights up and
                        # HAM sees real activity (K=1 fillers don't count)
                        # and releases the clock throttle early.
                        ps_wu = ps_projv.tile([P, CH], F32, tag="psV", name="ps_wu")
                        nc.tensor.matmul(
                            ps_wu[:, 0:hd], wq_f[0][:, 0:P], wq_f[0][:, :]
                        )

                    # solid warm-up burst: ~3.5us of continuous PE activity
                    # takes HAM through a full SHORT window so the real
                    # projection matmuls run at 2.4GHz as soon as possible
                    for wu in range(16):
                        dummy_mm()
                    xtiles = {}
                    for c in range(nch):
                        xtiles[c] = (
                            xpool.tile([P, nf, CH], F16, tag="xq", name="xq_t"),
                            xpool.tile([P, nf, CH], F16, tag="xkv", name="xkv_t"),
                        )

                    def issue_x_dma(c):
                        xq_t, xkv_t = xtiles[c]
                        if c == 0:
                            # interleaved quarter-DMAs so both the Q and K
                            # passes can start on partial chunks instead of
                            # waiting for the full 2MB per stream
                            for qd in range(4):
                                ftsl = slice(2 * qd, 2 * qd + 2)
                                nc.sync.dma_start(
                                    xq_t[:, ftsl, :], xqT_t[:, ftsl, c * CH : (c + 1) * CH]
                                )
                                nc.sync.dma_start(
                                    xkv_t[:, ftsl, :], xkvT_t[:, ftsl, c * CH : (c + 1) * CH]
                                )
                        else:
                            nc.sync.dma_start(xq_t[:], xqT_t[:, :, c * CH : (c + 1) * CH])
                            nc.sync.dma_start(xkv_t[:], xkvT_t[:, :, c * CH : (c + 1) * CH])

                    for c in range(nch):
                        issue_x_dma(c)
                    for c in range(nch):
                        xq_t, xkv_t = xtiles[c]
                        # Q pass (K pass reuses the same PSUM tags)
                        psQ = [ps_projqk.tile([P, CH], F32, tag=f"psQK{m}", name="psQ") for m in range(npair)]
                        for ft in range(nf):
                            for m in range(npair):
                                nc.tensor.matmul(
                                    psQ[m][:],
                                    wq_f[ft][:, m * P : (m + 1) * P],
                                    xq_t[:, ft],
                                    start=(ft == 0),
                                    stop=(ft == nf - 1),
                                )
                            if ft % 2 == 1:
                                if prelist:
                                    drip(1)
                                elif c == 0:
                                    dummy_mm()
                        for m in range(npair):
                            nc.vector.tensor_copy(QT[m][c][:], psQ[m][:])
                        # K pass
                        psK = [ps_projqk.tile([P, CH], F32, tag=f"psQK{m}", name="psK") for m in range(npair)]
                        for ft in range(nf):
                            for m in range(npair):
                                nc.tensor.matmul(
                                    psK[m][:],
                                    wk_f[ft][:, m * P : (m + 1) * P],
                                    xkv_t[:, ft],
                                    start=(ft == 0),
                                    stop=(ft == nf - 1),
                                )
                            if ft % 2 == 1:
                                if prelist:
                                    drip(1)
                                elif c == 0:
                                    dummy_mm()
                        for m in range(npair):
                            nc.vector.tensor_copy(KT[m][c][:], psK[m][:])
                        prelist.extend(PREFILL[c])
                        drip(2)
                        # V pass (xkv chunk tile as lhsT); one PSUM
                        # accumulation group per bank, so st is outer
                        for st in range(4):
                            psV = ps_projv.tile([P, CH], F32, tag="psV", name="psV")
                            for ft in range(nf):
                                nc.tensor.matmul(
                                    psV[:, 0:hd],
                                    xkv_t[:, ft, st * P : (st + 1) * P],
                                    wv_sb[:, ft, :],
                                    start=(ft == 0),
                                    stop=(ft == nf - 1),
                                )
                            kt = c * 4 + st
                            nc.vector.tensor_copy(
                                V[kt][:, :, 0:DH],
                                psV[:, 0:hd].rearrange("p (h d) -> p h d", h=hpc),
                            )
                            drip(1)
                        drip(2)
                    # whatever prefill remains flows out before attention
                    drip(len(prelist))

                # deferred work queue: sub-microsecond units injected into
                # later kt iterations so no engine sees a burst
                pending = []
                ps_opool = []  # filled once the attention-phase PSUM pool opens

                def queue_normalize(p_, c, stages, tail=False):
                    def emit(h01):
                        stage, den_r = stages[h01]
                        if tail:
                            # tail blocks: reciprocate the den row then
                            # broadcast it with a K=1 matmul into a (by now
                            # idle) scores PSUM slot — a much shorter chain
                            # than the GPSIMD broadcast, off the last-block
                            # critical path
                            rdr = npool.tile([1, CH], F32, tag=f"rdr{h01}", name="rdr")
                            nc.vector.reciprocal_approx_fast(out=rdr[:], in_=den_r[:])
                            rdr16 = npool.tile([1, CH], F16, tag=f"rdr16{h01}", name="rdr16")
                            nc.vector.tensor_copy(rdr16[:], rdr[:])
                            psB = ps_att.tile([P, 2 * CH], F32, tag="ps_s", name="psB")
                            nc.tensor.matmul(
                                psB[0:DH, 0:CH], ones_sb[0:1, 0:DH], rdr16[0:1, :]
                            )
                            nc.vector.tensor_tensor(
                                YT[p_][c][h01 * DH : (h01 + 1) * DH, :],
                                stage[:],
                                psB[0:DH, 0:CH],
                                mybir.AluOpType.mult,
                            )
                            return
                        # broadcast the denominator row on the otherwise-idle
                        # GPSIMD engine (source must sit at partition 0)
                        bc_sb = npool.tile([DH, CH], F32, tag=f"bc{h01}", name="bc_sb")
                        nc.gpsimd.partition_broadcast(bc_sb[:], den_r[:])
                        inv_sb = npool.tile([DH, CH], F32, tag=f"inv{h01}", name="inv_sb")
                        nc.vector.reciprocal_approx_fast(out=inv_sb[:], in_=bc_sb[:])
                        nc.vector.tensor_tensor(
                            YT[p_][c][h01 * DH : (h01 + 1) * DH, :],
                            stage[:],
                            inv_sb[:],
                            mybir.AluOpType.mult,
                        )

                    pending.append(lambda: emit(0))
                    pending.append(lambda: emit(1))

                def queue_outproj(c, tail=False):
                    for st in range(4):
                        qt = c * 4 + st

                        if tail:
                            # tail chunks pop after the exp stream has ended:
                            # the scores PSUM slots are idle, so both halves
                            # land in one wide 2-bank tile and the single
                            # rotating pool overlaps copy with next matmuls
                            def emit_st(st=st, c=c, qt=qt):
                                ps_w = ps_att.tile([P, 2 * CH], F32, tag="ps_s", name="ps_w")
                                for j in range(2):
                                    for m in range(hd // P):
                                        nc.tensor.matmul(
                                            ps_w[:, j * CH : (j + 1) * CH],
                                            YT[m][c][:, st * P : (st + 1) * P],
                                            wo_sb[:, m, j * CH : (j + 1) * CH],
                                            start=(m == 0),
                                            stop=(m == hd // P - 1),
                                        )
                                out_sb = opool.tile([P, o], F16, tag="out_sb", name="out_sb")
                                nc.vector.tensor_copy(out_sb[:], ps_w[:])
                                nc.sync.dma_start(
                                    out.ap()[qt * P : (qt + 1) * P, :], out_sb[:]
                                )

                            pending.append(emit_st)
                            continue

                        carrier = {}

                        def emit_half(j, st=st, c=c, carrier=carrier):
                            if j == 0:
                                carrier["out_sb"] = opool.tile([P, o], F16, tag="out_sb", name="out_sb")
                            ps_o = ps_opool[0].tile([P, CH], F32, tag="ps_o", name="ps_o")
                            for m in range(hd // P):
                                nc.tensor.matmul(
                                    ps_o[:],
                                    YT[m][c][:, st * P : (st + 1) * P],
                                    wo_sb[:, m, j * CH : (j + 1) * CH],
                                    start=(m == 0),
                                    stop=(m == hd // P - 1),
                                )
                            nc.vector.tensor_copy(
                                carrier["out_sb"][:, j * CH : (j + 1) * CH], ps_o[:]
                            )

                        def emit_dma(qt=qt, carrier=carrier):
                            nc.sync.dma_start(
                                out.ap()[qt * P : (qt + 1) * P, :], carrier["out_sb"][:]
                            )

                        pending.append(lambda f_=emit_half: f_(0))
                        pending.append(lambda f_=emit_half: f_(1))
                        pending.append(emit_dma)

                # ---- attention blocks ------------------------------------
                # blocks 0-2 fully prefilled; during block bi, emit
                # (bi+3, kt) right after this unit's AVs free the pt slot
                with (
                    tc.tile_pool(name="ps_y", bufs=3, space="PSUM") as ps_ypool,
                    tc.tile_pool(name="ps_o", bufs=1, space="PSUM") as ps_opool_,
                ):
                    ps_opool.append(ps_opool_)
                    for bi, (c, p_) in enumerate(blocks):
                        hA, hB = 2 * p_, 2 * p_ + 1
                        psY = [
                            ps_ypool.tile([DH + 1, CH], F32, tag="psY", name="psY_a"),
                            ps_ypool.tile([DH + 1, CH], F32, tag="psY", name="psY_b"),
                        ]
                        for kt in range(nkt):
                            pt = PT.pop((bi, kt))
                            nc.tensor.matmul(
                                psY[0][:],
                                V[kt][:, hA, :],
                                pt[:, 0:CH],
                                start=(kt == 0),
                                stop=(kt == nkt - 1),
                            )
                            nc.tensor.matmul(
                                psY[1][:],
                                V[kt][:, hB, :],
                                pt[:, CH : 2 * CH],
                                start=(kt == 0),
                                stop=(kt == nkt - 1),
                            )
                            if pending:
                                pending.pop(0)()
                            if bi + 3 < nblk:
                                emit_score_exp(bi + 3, kt)
                        # stage psY out to SBUF immediately so the PSUM banks
                        # free up for the next block's AV matmuls
                        stages = []
                        for h01 in range(2):
                            stage = npool.tile(
                                [DH, CH], F32, tag=f"stage{h01}", name="stage"
                            )
                            nc.vector.tensor_copy(stage[:], psY[h01][0:DH, :])
                            den_r = npool.tile([1, CH], F32, tag=f"den{h01}", name="den_r")
                            nc.vector.tensor_copy(den_r[:], psY[h01][DH : DH + 1, :])
                            stages.append((stage, den_r))
                        queue_normalize(p_, c, stages, tail=(bi >= nblk - 3))
                        if p_ == npair - 1:
                            queue_outproj(c, tail=(c >= nch - 2))
                    while pending:
                        pending.pop(0)()

    nc.compile()
    return nc


def make_in_maps(inputs_q, inputs_kv, wq, wk, wv, wo):
    """Shard full inputs into 8 per-core input dicts (host-side)."""
    in_maps = []
    scale = 1.0 / np.sqrt(DH)
    for core in range(NCORES):
        b = core // (NCORES // B)
        hg = core % (NCORES // B)
        hs = slice(hg * HPC, (hg + 1) * HPC)
        in_maps.append(
            {
                "xqT": np.ascontiguousarray(inputs_q[b].T).astype(np.float16),
                "xkvT": np.ascontiguousarray(inputs_kv[b].T).astype(np.float16),
                "wq": np.ascontiguousarray(
                    (wq[:, hs, :] * scale).reshape(F, HPC * DH)
                ).astype(np.float16),
                "wk": np.ascontiguousarray(wk[:, hs, :].reshape(F, HPC * DH)).astype(
                    np.float16
                ),
                "wv": np.ascontiguousarray(wv[:, hs, :].reshape(F, HPC * DH)).astype(
                    np.float16
                ),
                "wo": np.ascontiguousarray(wo[hs].reshape(HPC * DH, O)).astype(
                    np.float16
                ),
            }
        )
    return in_maps


_CACHE = {}


def _get_program():
    if "nc" not in _CACHE:
        _CACHE["nc"] = build_program()
    return _CACHE["nc"]


def run_sharded(inputs_q, inputs_kv, wq, wk, wv, wo, bo, **spmd_kwargs):
    """Build in_maps, run on 8 cores, reduce partials. Returns (out, results)."""
    nc = _get_program()
    in_maps = make_in_maps(inputs_q, inputs_kv, wq, wk, wv, wo)
    res = run_bass_kernel_spmd(nc, in_maps, core_ids=list(range(NCORES)), **spmd_kwargs)
    gpb = NCORES // B  # head-group cores per batch element
    out = np.zeros((B, S, O), dtype=np.float32)
    for core in range(NCORES):
        out[core // gpb] += res.results[core]["out"].astype(np.float32)
    out += np.asarray(bo, dtype=np.float32)
    return out, res


def kernel(inputs_q, inputs_kv, wq, wk, wv, wo, bo):
    out, _ = run_sharded(
        np.asarray(inputs_q),
        np.asarray(inputs_kv),
        np.asarray(wq),
        np.asarray(wk),
        np.asarray(wv),
        np.asarray(wo),
        np.asarray(bo),
    )
    return out
